# revision 17
# baseline (speedup 1.0000x reference)
"""ConvHex GNN message-passing kernel for 8 Trainium2 NeuronCores.

Math (per reference):
  xt = x.transpose -> [B, N, C]
  out[b,o,n] = (sum_c wc[o,c]*xt[b,n,c]
                + sum_k sum_c wk[o,c,k]*xt[b,nb[n,k],c]*valid) / tv + bias[o]
  tv = (neighbors[0] >= 0).sum() + 1

Sharding: 8 cores = 4 batch-groups (8 batches each) x 2 n-halves (10000 each).
Per core the full-N token table (bf16, 8 batches x 32 ch = 256 features = 512 B
per token) lives in SBUF; neighbor features are fetched with SBUF-source
transpose dma_gather (features land on partitions, indices on the free dim),
then block-diagonal bf16 matmuls (contraction = 4 batches x 32 ch) accumulate
center + 6 neighbor taps into PSUM in fp32. Invalid neighbors are pointed at a
zeroed pad token, so no masking is needed on-device.
"""

import contextlib
import os

import numpy as np
import ml_dtypes

import concourse.mybir as mybir
import concourse.tile as tile
from concourse import bacc
from concourse.bass_utils import run_bass_kernel_spmd

# Problem shape (hardcoded per contract)
B, C_IN, C_OUT, N, K = 32, 32, 64, 20000, 6

P = 128
BG = 8                 # batches per core
NH = 10000             # hexagons per n-half
NHP = 10240            # padded (20 chunks of 512)
CHUNK = 512
NCHUNK = NHP // CHUNK  # 20
REG = 1024             # gather granularity (indices per dma_gather)
NREG = NHP // REG      # 10
ELEM = 2 * P           # bf16 features per token row (512 B)
NRANK = 157            # ceil(20001 / 128) -> token capacity 20096
PAD_TOK = N            # index of the all-zero pad token
TOKF = NRANK * ELEM    # token tile free dim (bf16 elems per partition)
NGRP = K + 1           # center + 6 neighbor taps

_BF16 = ml_dtypes.bfloat16

_nc_cache = None
_last_results = None  # BassKernelResults of the most recent run (for profiling)


def _build_nc(repeat=1):
    """Build the single-core Bass program (SPMD across 8 cores).

    repeat > 1 wraps the steady-state pipeline in a hardware loop that
    re-runs it `repeat` times — used only for wall-clock delta timing
    (the per-iteration time is (t_repeat - t_1) / (repeat - 1)).
    """
    nc = bacc.Bacc("TRN2", debug=False)

    tok_hbm = nc.dram_tensor("tok", [P, TOKF], mybir.dt.bfloat16, kind="ExternalInput")
    cmaj_hbm = nc.dram_tensor(
        "cmaj", [2, P, NHP], mybir.dt.bfloat16, kind="ExternalInput"
    )
    idx_hbm = nc.dram_tensor(
        "idx", [P, K * (NHP // 16)], mybir.dt.int16, kind="ExternalInput"
    )
    wts_hbm = nc.dram_tensor(
        "wts", [P, NGRP * 2 * P], mybir.dt.bfloat16, kind="ExternalInput"
    )
    bias_hbm = nc.dram_tensor("biast", [P, 1], mybir.dt.float32, kind="ExternalInput")
    out_hbm = nc.dram_tensor(
        "out", [4 * P, NHP], mybir.dt.float32, kind="ExternalOutput"
    )
    out_v = out_hbm[:, :].rearrange("(bp p) n -> p bp n", p=P)

    idx_cols = NHP // 16  # 640 per k

    with tile.TileContext(nc) as tc:
        with (
            tc.tile_pool(name="persist", bufs=1) as pp,
            tc.tile_pool(name="io", bufs=2) as iop,
            tc.tile_pool(name="psum", bufs=2, space="PSUM") as psp,
        ):
            tok_sb = pp.tile([P, TOKF], mybir.dt.bfloat16)
            idx_sb = pp.tile([P, K * idx_cols], mybir.dt.int16)
            wts_sb = pp.tile([P, NGRP * 2 * P], mybir.dt.bfloat16)
            bias_sb = pp.tile([P, 1], mybir.dt.float32)
            # chunk the token load: one contiguous per-partition run must stay
            # well under the 64 KB DMA descriptor length limit
            step = 8192  # bf16 elems (16 KB) per partition per DMA
            for o in range(0, TOKF, step):
                e = min(o + step, TOKF)
                nc.sync.dma_start(tok_sb[:, o:e], tok_hbm[:, o:e])
            nc.sync.dma_start(idx_sb[:], idx_hbm[:, :])
            nc.sync.dma_start(wts_sb[:], wts_hbm[:, :])
            nc.sync.dma_start(bias_sb[:], bias_hbm[:, :])
            nreg = nc.gpsimd.to_reg(REG)  # shared num_idxs register

            rep_ctx = (
                tc.For_i(0, repeat, 1) if repeat > 1 else contextlib.nullcontext()
            )
            with rep_ctx:
                _emit_body(nc, tc, iop, psp, tok_sb, idx_sb, wts_sb, bias_sb,
                           nreg, out_v, idx_cols, cmaj_hbm)
    nc.compile()
    return nc


def _emit_body(nc, tc, iop, psp, tok_sb, idx_sb, wts_sb, bias_sb, nreg, out_v,
               idx_cols, cmaj_hbm):
    for r in range(NREG):
        gt = [iop.tile([P, 2 * REG], mybir.dt.bfloat16, tag=f"g{k}",
                       name=f"g{k}_{r}") for k in range(K)]
        for k in range(K):
            nc.gpsimd.dma_gather(
                gt[k][:].rearrange("p (q g) -> p q g", g=REG),
                tok_sb[:],
                idx_sb[:, k * idx_cols + r * (REG // 16):
                       k * idx_cols + (r + 1) * (REG // 16)],
                REG,
                nreg,
                ELEM,
                transpose=True,
                single_packet=False,
                sbuf_tokens_per_rank=P,
                sbuf_free_dim_per_rank=ELEM * 2,
            )
        cm = [iop.tile([P, REG], mybir.dt.bfloat16, tag=f"cm{q}",
                       name=f"cm{q}_{r}") for q in range(2)]
        for q in range(2):
            nc.sync.dma_start(cm[q][:], cmaj_hbm[q, :, r * REG:(r + 1) * REG])

        for tt in range(2):
            t = 2 * r + tt
            ps = [psp.tile([P, CHUNK], mybir.dt.float32, tag=f"ps{bp}",
                           name=f"ps{bp}_{t}") for bp in range(4)]
            for par in range(2):
                for g in range(NGRP):
                    lhsT = wts_sb[:, (g * 2 + par) * P:(g * 2 + par + 1) * P]
                    for q in range(2):
                        if g == 0:
                            rhs = cm[q][:, tt * CHUNK:(tt + 1) * CHUNK]
                        else:
                            base = q * REG + tt * CHUNK
                            rhs = gt[g - 1][:, base:base + CHUNK]
                        nc.tensor.matmul(
                            ps[2 * q + par][:],
                            lhsT,
                            rhs,
                            start=(g == 0),
                            stop=(g == NGRP - 1),
                        )
            stage = iop.tile([P, 4 * CHUNK], mybir.dt.float32, tag="stage",
                             name=f"stage_{t}")
            for bp in range(4):
                nc.vector.tensor_scalar_add(
                    stage[:, bp * CHUNK:(bp + 1) * CHUNK],
                    ps[bp][:],
                    bias_sb[:, :1],
                )
            w = CHUNK if t < NCHUNK - 1 else NH - (NCHUNK - 1) * CHUNK
            nc.sync.dma_start(
                out_v[:, :, t * CHUNK:t * CHUNK + w],
                stage[:].rearrange("p (bp c) -> p bp c", bp=4)[:, :, :w],
            )


def _prep_core_inputs(x_bf, neighbors, wts_np, bias_np):
    """Host-side sharding/layout prep. Returns list of 8 in_maps."""
    # idx per half: [10240, 6] int16 with invalid/pad -> PAD_TOK,
    # wrapped to [16, 640] per k (i at [i%16, i//16]), replicated 8x.
    idx_h = []
    for h in range(2):
        nb = neighbors[h * NH:(h + 1) * NH].astype(np.int64)
        v = np.where(nb >= 0, nb, PAD_TOK)
        v = np.concatenate(
            [v, np.full((NHP - NH, K), PAD_TOK, dtype=np.int64)], axis=0
        )
        per_k = []
        for k in range(K):
            w = v[:, k].reshape(NHP // 16, 16).T.astype(np.int16)  # [16, 640]
            per_k.append(np.tile(w, (8, 1)))                        # [128, 640]
        idx_h.append(np.concatenate(per_k, axis=1))                 # [128, 3840]

    in_maps = []
    for core in range(8):
        g, h = core % 4, core // 4
        xg = x_bf[BG * g:BG * (g + 1)]                      # [8, 32, 20000]

        tok = np.zeros((NRANK * P, 2 * P), dtype=_BF16)
        tok[:N] = np.transpose(xg, (2, 0, 1)).reshape(N, 2 * P)
        tok = np.ascontiguousarray(
            tok.reshape(NRANK, P, 2 * P).transpose(1, 0, 2)
        ).reshape(P, TOKF)

        cmaj = np.zeros((2, P, NHP), dtype=_BF16)
        cmaj[:, :, :NH] = xg[:, :, h * NH:(h + 1) * NH].reshape(2, P, NH)

        in_maps.append({
            "tok": tok,
            "cmaj": cmaj,
            "idx": idx_h[h],
            "wts": wts_np,
            "biast": bias_np,
        })
    return in_maps


def kernel(x, neighbors, weight_center, weight_neighbors, bias):
    global _nc_cache
    x = np.asarray(x)
    neighbors = np.asarray(neighbors)
    weight_center = np.asarray(weight_center, dtype=np.float32)
    weight_neighbors = np.asarray(weight_neighbors, dtype=np.float32)
    bias = np.asarray(bias, dtype=np.float32)

    tv = np.float32((np.asarray(neighbors[0]) >= 0).sum() + 1)

    # Block-diagonal weights [128, 14*128]: for tap g and parity par,
    # W[b4*32+c, b2*64+o] = wg[o,c]/tv iff b4 == 2*par + b2.
    wblk = np.zeros((NGRP, 2, 4, C_IN, 2, C_OUT), dtype=np.float32)
    for g in range(NGRP):
        wg = weight_center if g == 0 else weight_neighbors[:, :, g - 1]
        wgt = (wg / tv).T  # [c, o]
        for par in range(2):
            for b2 in range(2):
                wblk[g, par, 2 * par + b2, :, b2, :] = wgt
    wts_np = np.ascontiguousarray(
        wblk.reshape(NGRP * 2, P, P).transpose(1, 0, 2)
    ).reshape(P, NGRP * 2 * P).astype(_BF16)

    bias_np = np.ascontiguousarray(
        np.tile(bias.reshape(1, C_OUT), (1, 2)).reshape(P, 1)
    ).astype(np.float32)

    x_bf = x.astype(_BF16)
    in_maps = _prep_core_inputs(x_bf, neighbors, wts_np, bias_np)

    if _nc_cache is None:
        _nc_cache = _build_nc()
    res = run_bass_kernel_spmd(
        _nc_cache,
        in_maps,
        core_ids=list(range(8)),
        trace=bool(os.environ.get("CONVHEX_TRACE")),
    )
    global _last_results
    _last_results = res

    out = np.empty((B, C_OUT, N), dtype=np.float32)
    for core in range(8):
        g, h = core % 4, core // 4
        oc = res.results[core]["out"].reshape(BG, C_OUT, NHP)
        out[BG * g:BG * (g + 1), :, h * NH:(h + 1) * NH] = oc[:, :, :NH]
    return out
